# revision 19
# baseline (speedup 1.0000x reference)
"""Trainium2 Bass kernel for nn_DigitConvolutionalModel (dense_cnn).

Math: the 3x3 valid conv is linear in x, so it folds into fc1:
    conv(x) @ fc1_w.T == x @ (C @ fc1_w.T)  with C [784, 676] the conv matrix.
The whole model is then a 3-layer MLP:
    out = relu(relu(x @ W1 + b1) @ W2 + b2) @ W3 + b3
with W1 = C @ fc1_w.T [784,512], W2 = fc2_w.T [512,512], W3 = out_w.T [512,10].

Sharding: pure data parallelism; batch 32768 -> 8 cores x 4096 rows.

On-chip formulation is fully transposed (features on SBUF partitions, batch on
the free dim): each layer computes h^T = act(W_l as lhsT, rhs = h_{l-1}^T).

Two-phase schedule (v2): all 8 chunks' layer-1 matmuls first (only x + w1
needed early -> input DMA keeps up with the PE from the first k-tile), then
all layer-2/3 (zero DMA pressure; h1 for all chunks stays resident in SBUF,
32KB/partition). h1 activations on ACT, h2 on the otherwise-idle DVE
(tensor_scalar add-bias + max 0), logits bias on ACT.

Startup: 8 small dummy matmuls begin the PE HAM ramp at ~7.6us while the
first k-tile DMAs (x0 k0 on one HWDGE ring, w1 k0 on the other) land ~9.5us;
real layer-1 matmuls ride the remaining ramp. Chunk 0/1 x and w1 are sliced
per k-tile and interleaved across both rings in consumption order.

Per core: batch 4096 in 8 chunks of N=512 (one fp32 PSUM bank). Layer-1
K = 784 = 7 k-tiles of 112 partitions; layers 2/3 K = 512 = 4 k-tiles of 128.
Matmul dtype bf16 (fp32 PSUM): 2.5e-3 L2 rel err. PE floor: 384 matmuls x
~216ns = 83us.
"""

import numpy as np
import ml_dtypes

NCORES = 8
B = 32768
BC = B // NCORES  # rows per core
CH = 512          # batch chunk = matmul moving free dim = one fp32 PSUM bank
NCH = BC // CH
KP1, KT1 = 112, 7  # layer-1 contraction tiling: 784 = 7 * 112
MT1 = 4            # 512 out feats = 4 m-tiles of 128
KT2, MT2 = 4, 4    # layer-2: K=512, M=512
KT3, MO = 4, 10    # layer-3: K=512, M=10
N_WARM = 10        # small dummy matmuls to start the HAM ramp pre-data

MM_DTYPE = "bf16"

_cache = {}


def _np_dtype():
    return ml_dtypes.bfloat16 if MM_DTYPE == "bf16" else np.float32


def _build():
    """Trace + compile the Bass program once per process."""
    if "nc" in _cache:
        return _cache["nc"]

    from contextlib import ExitStack

    import concourse.bass as bass
    import concourse.tile as tile
    from concourse import bacc, mybir
    from concourse.bass import ts, ds

    DT = mybir.dt.bfloat16 if MM_DTYPE == "bf16" else mybir.dt.float32r
    F32 = mybir.dt.float32
    Relu = mybir.ActivationFunctionType.Relu
    Ident = mybir.ActivationFunctionType.Identity

    from concourse.vector_clock import ScopedClock

    class _FastExitTileContext(tile.TileContext):
        """Skip the exit semaphore-clear chain + second barrier (~2us tail).

        Safety: re-execution of the loaded NEFF is exercised by repeated
        kernel() calls; verified bitwise-identical with this override."""

        def _drain_and_barrier(self, tick_clock, wait_clock):
            drain_inst = self.nc.sync.drain()
            wait_clock.add_sem_waits(
                drain_inst.ins, ScopedClock({None: tick_clock.global_clock})
            )
            popped = self.nc._tile_sem_poison_stack.pop()
            assert popped is self._sem_poison

    nc = bacc.Bacc(
        "TRN2",
        target_bir_lowering=False,
        debug=False,
        enable_asserts=False,
        num_devices=NCORES,
        enable_partition_id=False,
    )

    # Layouts match SBUF tiles exactly -> per-partition contiguous runs.
    xt_d = nc.dram_tensor("xt", [NCH, KP1, KT1 * CH], DT, kind="ExternalInput")
    w1_d = nc.dram_tensor("w1", [KP1, KT1 * 512], DT, kind="ExternalInput")
    w2_d = nc.dram_tensor("w2", [128, KT2 * 512], DT, kind="ExternalInput")
    w3_d = nc.dram_tensor("w3", [128, KT3 * MO], DT, kind="ExternalInput")
    b_d = nc.dram_tensor("b", [128, MT1 + MT2 + 1], F32, kind="ExternalInput")
    out_d = nc.dram_tensor("out", [MO, BC], F32, kind="ExternalOutput")

    with _FastExitTileContext(nc) as tc, ExitStack() as ctx:
        consts = ctx.enter_context(tc.tile_pool(name="consts", bufs=1))
        h1_pool = ctx.enter_context(tc.tile_pool(name="h1", bufs=4 * NCH))
        h2_pool = ctx.enter_context(tc.tile_pool(name="h2", bufs=12))
        oc_pool = ctx.enter_context(tc.tile_pool(name="oc", bufs=2))
        # layer-3 psums share the ps1 pool (idle once phase 1 drains), so
        # no dedicated ps3 bank and ps2 gets 3 banks: 5 + 3 = 8.
        ps1 = ctx.enter_context(tc.tile_pool(name="ps1", bufs=5, space="PSUM"))
        ps2 = ctx.enter_context(tc.tile_pool(name="ps2", bufs=3, space="PSUM"))

        # --- PE HAM-ramp warm-up: small dummies while the first DMAs fly ---
        warm_sb = consts.tile([128, 256], DT, name="warm_sb")
        nc.gpsimd.memset(warm_sb[:], 0.0)
        warm_ps = ps1.tile([128, 256], F32, name="warm_ps", tag="ps1")
        for _ in range(N_WARM):
            nc.tensor.matmul(
                warm_ps[:], warm_sb[:, :128], warm_sb[:], start=True, stop=True
            )

        # --- input DMAs: both HWDGE rings, interleaved in consumption order.
        # scalar ring: x0 per-k, x1 k0-3, x2/x3/x4 k0-3, x5.
        # sync ring:   w1 per-k, b, x1 k4-6, x2/x3/x4 k4-6, x6, x7, w2, w3.
        x0p = {}  # piece key -> tile
        for key, lo, hi in (("k0", 0, 1), ("k1", 1, 2), ("k2", 2, 3),
                            ("k34", 3, 5), ("k56", 5, 7)):
            t = consts.tile([KP1, (hi - lo) * CH], DT, name=f"x0{key}")
            nc.scalar.dma_start(t[:], xt_d[0][:, lo * CH : hi * CH])
            w = consts.tile([KP1, (hi - lo) * 512], DT, name=f"w1{key}")
            nc.sync.dma_start(w[:], w1_d[:, lo * 512 : hi * 512])
            x0p[key] = (t, w, lo)

        b_sb = consts.tile([128, MT1 + MT2 + 1], F32, name="b_sb")
        nc.sync.dma_start(b_sb[:], b_d[:])

        x1p = {}
        for eng, key, lo, hi in (
            (nc.scalar, "k01", 0, 2), (nc.scalar, "k23", 2, 4),
            (nc.sync, "k45", 4, 6), (nc.sync, "k6", 6, 7),
        ):
            t = consts.tile([KP1, (hi - lo) * CH], DT, name=f"x1{key}")
            eng.dma_start(t[:], xt_d[1][:, lo * CH : hi * CH])
            x1p[key] = (t, lo)

        xab = {}  # chunks 2-4: k0-3 on scalar, k4-6 on sync
        for n in (2, 3, 4):
            ta = consts.tile([KP1, 4 * CH], DT, name=f"x{n}a")
            nc.scalar.dma_start(ta[:], xt_d[n][:, : 4 * CH])
            tb = consts.tile([KP1, 3 * CH], DT, name=f"x{n}b")
            nc.sync.dma_start(tb[:], xt_d[n][:, 4 * CH :])
            xab[n] = (ta, tb)

        xw = {}  # chunks 5-7 whole
        for n, eng in ((5, nc.scalar), (6, nc.sync), (7, nc.sync)):
            t = consts.tile([KP1, KT1 * CH], DT, name=f"x{n}")
            eng.dma_start(t[:], xt_d[n])
            xw[n] = t

        w2_sb = consts.tile([128, KT2 * 512], DT, name="w2_sb")
        nc.sync.dma_start(w2_sb[:], w2_d[:])
        w3_sb = consts.tile([128, KT3 * MO], DT, name="w3_sb")
        nc.sync.dma_start(w3_sb[:], w3_d[:])

        def w1s(ki, mi):
            if ki < 3:
                w = x0p[f"k{ki}"][1]
                return w[:, ds(mi * 128, 128)]
            key = "k34" if ki < 5 else "k56"
            w, lo = x0p[key][1], x0p[key][2]
            return w[:, ds((ki - lo) * 512 + mi * 128, 128)]

        def xsl(n, ki):
            if n == 0:
                if ki < 3:
                    return x0p[f"k{ki}"][0][:]
                key = "k34" if ki < 5 else "k56"
                t, _, lo = x0p[key]
                return t[:, ts(ki - lo, CH)]
            if n == 1:
                key = ("k01", "k01", "k23", "k23", "k45", "k45", "k6")[ki]
                t, lo = x1p[key]
                return t[:, ts(ki - lo, CH)]
            if n in xab:
                ta, tb = xab[n]
                if ki < 4:
                    return ta[:, ts(ki, CH)]
                return tb[:, ts(ki - 4, CH)]
            return xw[n][:, ts(ki, CH)]

        # ---------------- phase 1: layer 1 for all chunks ----------------
        h1t = [[None] * MT1 for _ in range(NCH)]
        for n in range(NCH):
            for mi in range(MT1):
                h1t[n][mi] = h1_pool.tile(
                    [128, CH], DT, name=f"h1_{n}_{mi}", tag="h1"
                )
        for n in range(NCH):
            ps = [
                ps1.tile([128, CH], F32, name=f"ps1_{n}_{mi}", tag="ps1")
                for mi in range(MT1)
            ]
            if n <= 1:
                # k-outer: consume k-tiles as their DMAs land (relaxes each
                # k-tile's deadline to +0.86us per tile); chunk 1 rotated so
                # the sync-ring pieces (which land first) go first
                korder = tuple(range(KT1)) if n == 0 else (4, 5, 6, 0, 1, 2, 3)
                for j, ki in enumerate(korder):
                    for mi in range(MT1):
                        nc.tensor.matmul(
                            ps[mi][:], w1s(ki, mi), xsl(n, ki),
                            start=(j == 0), stop=(j == KT1 - 1),
                        )
            else:
                # m-outer (spreads PSUM drains for the ps1 rotation)
                for mi in range(MT1):
                    for ki in range(KT1):
                        nc.tensor.matmul(
                            ps[mi][:], w1s(ki, mi), xsl(n, ki),
                            start=(ki == 0), stop=(ki == KT1 - 1),
                        )
            for mi in range(MT1):
                nc.scalar.activation(
                    h1t[n][mi][:], ps[mi][:], Relu, bias=b_sb[:, mi : mi + 1]
                )

        # ---------------- phase 2: layers 2+3 for all chunks ----------------
        h2t = [[None] * MT2 for _ in range(NCH)]
        Add = mybir.AluOpType.add
        Max = mybir.AluOpType.max

        def layer2(n):
            for mi in range(MT2):
                h2t[n][mi] = h2_pool.tile(
                    [128, CH], DT, name=f"h2_{n}_{mi}", tag="h2"
                )
                ps = ps2.tile([128, CH], F32, name=f"ps2_{n}_{mi}", tag="ps2")
                for ki in range(KT2):
                    nc.tensor.matmul(
                        ps[:],
                        w2_sb[:, ds(ki * 512 + mi * 128, 128)],
                        h1t[n][ki][:],
                        start=(ki == 0),
                        stop=(ki == KT2 - 1),
                    )
                # relu(ps + b2): alternate DVE / ACT so neither engine's
                # queue gates the ps2 pool rotation
                if mi % 2 == 0:
                    nc.vector.tensor_scalar(
                        out=h2t[n][mi][:],
                        in0=ps[:],
                        scalar1=b_sb[:, MT1 + mi : MT1 + mi + 1],
                        scalar2=0.0,
                        op0=Add,
                        op1=Max,
                    )
                else:
                    nc.scalar.activation(
                        h2t[n][mi][:],
                        ps[:],
                        Relu,
                        bias=b_sb[:, MT1 + mi : MT1 + mi + 1],
                    )

        def layer3(n, halves=1):
            ps = ps1.tile([MO, CH], F32, name=f"ps3_{n}", tag="ps1")
            for ki in range(KT3):
                nc.tensor.matmul(
                    ps[:],
                    w3_sb[:, ts(ki, MO)],
                    h2t[n][ki][:],
                    start=(ki == 0),
                    stop=(ki == KT3 - 1),
                )
            oct_ = oc_pool.tile([MO, CH], F32, name=f"oc_{n}", tag="oc")
            bia = b_sb[:MO, MT1 + MT2 : MT1 + MT2 + 1]
            if n in (NCH - 3, NCH - 2):
                # keep the serial ACT queue clear so the final chunk's oc
                # isn't queued behind these two; DVE is idle here
                nc.vector.tensor_scalar(
                    out=oct_[:], in0=ps[:], scalar1=bia, scalar2=None, op0=Add
                )
            else:
                nc.scalar.activation(oct_[:], ps[:], Ident, bias=bia)
            nc.sync.dma_start(out_d[:, ts(n, CH)], oct_[:])

        # L3(n) lags two chunks behind L2 so the h2 activations (ACT/DVE)
        # are never on the PE's critical path, including for the last chunk
        for n in range(NCH):
            layer2(n)
            if n >= 2:
                layer3(n - 2)
        layer3(NCH - 2)
        layer3(NCH - 1)

    nc.compile()
    _cache["nc"] = nc
    return nc


def _prep_inputs(x, conv_w, fc1_w, fc1_b, fc2_w, fc2_b, out_w, out_b):
    dt = _np_dtype()
    f32 = np.float32

    # Conv as a [784, 676] matrix (exact in fp64), folded into fc1.
    C = np.zeros((784, 676), dtype=np.float64)
    oy, ox = np.meshgrid(np.arange(26), np.arange(26), indexing="ij")
    cols = (oy * 26 + ox).ravel()
    for ky in range(3):
        for kx in range(3):
            rows = ((oy + ky) * 28 + (ox + kx)).ravel()
            np.add.at(C, (rows, cols), float(conv_w[ky, kx]))
    W1 = (C @ fc1_w.T.astype(np.float64)).astype(f32)  # [784, 512]

    # [p, t*m] layouts: one contiguous run per SBUF partition
    w1 = np.ascontiguousarray(
        W1.reshape(KT1, KP1, 512).transpose(1, 0, 2)
    ).reshape(KP1, KT1 * 512).astype(dt)
    w2 = np.ascontiguousarray(
        np.ascontiguousarray(fc2_w.T).reshape(KT2, 128, 512).transpose(1, 0, 2)
    ).reshape(128, KT2 * 512).astype(dt)
    # [512,10] -> [4,128,10] -> [128, 4*10] so each partition is one 80B run
    w3 = np.ascontiguousarray(
        np.ascontiguousarray(out_w.T).reshape(KT3, 128, MO).transpose(1, 0, 2)
    ).reshape(128, KT3 * MO).astype(dt)
    b3col = np.zeros((128, 1), dtype=np.float64)
    b3col[:MO, 0] = out_b
    b = np.ascontiguousarray(
        np.concatenate(
            [fc1_b.reshape(MT1, 128).T, fc2_b.reshape(MT2, 128).T, b3col],
            axis=1,
        )
    ).astype(f32)

    in_maps = []
    for c in range(NCORES):
        xc = x[c * BC : (c + 1) * BC].T.astype(dt, order="C")  # [784, BC]
        # -> [chunk, p, t*ch]: chunk DMA is one 7KB run per partition
        xch = np.ascontiguousarray(
            xc.reshape(KT1, KP1, NCH, CH).transpose(2, 1, 0, 3)
        ).reshape(NCH, KP1, KT1 * CH)
        in_maps.append(
            {
                "xt": xch,
                "w1": w1,
                "w2": w2,
                "w3": w3,
                "b": b,
            }
        )
    return in_maps


def kernel(x, conv_w, fc1_w, fc1_b, fc2_w, fc2_b, out_w, out_b, _results=None):
    from concourse.bass_utils import run_bass_kernel_spmd

    # Inputs may arrive as jax arrays; do all host prep in numpy.
    x, conv_w, fc1_w, fc1_b, fc2_w, fc2_b, out_w, out_b = (
        np.asarray(a)
        for a in (x, conv_w, fc1_w, fc1_b, fc2_w, fc2_b, out_w, out_b)
    )
    nc = _build()
    in_maps = _prep_inputs(x, conv_w, fc1_w, fc1_b, fc2_w, fc2_b, out_w, out_b)
    res = run_bass_kernel_spmd(nc, in_maps, core_ids=list(range(NCORES)))
    if _results is not None:
        _results.append(res)
    out = np.empty((B, 10), dtype=np.float32)
    for c in range(NCORES):
        out[c * BC : (c + 1) * BC, :] = res.results[c]["out"].T
    return out
